# revision 79
# baseline (speedup 1.0000x reference)
"""Causal self-attention (B=4, T=2048, C=1024, H=16, D=64) on 8 TRN2 NeuronCores.

Sharding: core = 2*b + g  (b = batch 0..3, g = head-group 0..1; heads 8g..8g+7).
Each core computes, for its batch b and its 8 heads:
  qkv projection, causal softmax attention, and a PARTIAL output projection
  (its 512 rows of W_proj). Host sums the two partials per batch (+ b_proj).

Final design (301.7us baseline -> 238.2us, rel err 8e-3 vs 2e-2 budget):
  - q/k/v projections in fp8e4m3 DoubleRow matmuls (0.5 cyc/output-col,
    contraction 256/instr) with an error-compensated 3-term split:
    x*w = x8*w8 + (x8/16)*(16rw) + (16rx)*(w8/16)  [term i = array i of
    both operands; x side ships (x8, x8/16, 16rx), w side (w8, 16rw,
    w8/16)]. Drops only the rx*rw term (~eps^2) -> bf16-class accuracy at
    0.75x the bf16 PE cost. Attention S/AV and out-proj stay bf16 (1 cyc/
    col; S contraction is 64 so DoubleRow cannot apply; fp8 P/V would blow
    the error budget).
  - softmax denominator for free: AV lhsT = [V_h | ones64] (128 cols), so
    PSUM rows 64:127 = sum_k P replicated across 64 partitions. Normalize =
    DVE reciprocal reading rows 64:128 and writing rows 0:64 (engine lanes
    map RELATIVELY within the AP partition range, verified on HW) + one DVE
    multiply vs PSUM rows 0:64. Only ONE psum operand per vector op is
    legal; GPSIMD cannot touch PSUM at all.
  - consolidated DMAs: transfers run FIFO in descriptor order at ~350 B/ns
    with ~625ns serial descriptor gen per queue, so emission order IS the
    transfer schedule; spans ordered by first consumption.
  - chunk-outer schedule with a filler queue: V(t0..7); {qk(p,ch0);
    att(p,c0) + V(t8..15) fillers} x4; {qk(p,ch1); att(p,c1) + out-proj
    qtile 0..7 + qk(3,ch1) fillers} x4; out-proj 8..15. The exp stream on
    ACT (0.83ns/col, ~152us) is the attention-phase co-bottleneck; popping
    one independent PE op per ki keeps PE fed through exp latency. Fillers
    and direct emitters share the "small" psum tag, so the queue must fully
    drain before any direct allocation (buffer rotation would corrupt an
    in-flight accumulation).
  - 2-deep software pipeline inside a head-chunk: S(0),S(1),S(2) precede
    AV(0), giving chunk boundaries ~2.5us of PE runway to hide the previous
    chunk's ytps drain (rcp+mul, ~2.6us DVE).
  - PE warmup: 12 throwaway matmuls during the initial DMA wait ramp the PE
    p-state (1.54 -> 0.42 ns/col) before real work arrives.
  - PSUM banks (8): stp [128,1024] bufs=2 (4) + ytps [128,1024] (2) +
    small [128,512] bufs=2 (2).
  - psum->SBUF copy engines chosen to keep FIFOs clear: qk q-side DVE,
    k-side ACT Identity (per-partition bias AP); out-proj tail alternates
    DVE/ACT per qtile with split DMAs; V copies DVE; v-ones memsets
    per-tile on DVE (strided).
"""

import sys

try:
    import concourse  # noqa: F401
except ImportError:
    sys.path.insert(0, "/opt/trn_rl_repo")

import numpy as np
import ml_dtypes

import concourse.bacc as bacc
import concourse.mybir as mybir
import concourse.tile as tile

F32 = mybir.dt.float32
BF16 = mybir.dt.bfloat16
AF = mybir.ActivationFunctionType
ALU = mybir.AluOpType

B, T, C = 4, 2048, 1024
H, D = 16, 64
NCORES = 8
HL = 8          # heads per core (local)
NPAIR = 4       # head pairs per core
CH = 1024       # query chunk
NCH = T // CH   # 2
KT = T // 128   # 16 key tiles
CT = C // 128   # 8 contraction tiles over C
SCALE = 1.0 / 8.0  # 1/sqrt(D)

NPBF16 = ml_dtypes.bfloat16
NPF8 = ml_dtypes.float8_e4m3

_prog_cache = {}


def build_program(debug=False):
    key = debug
    if key in _prog_cache:
        return _prog_cache[key]

    nc = bacc.Bacc(None, target_bir_lowering=False, debug=debug)

    # fp8 error-compensated triplets: x*w = x8*w8 + (x8/16)*(16rw) +
    # (16rx)*(w8/16); term t multiplies array t of both sides. Each term
    # runs as a DoubleRow fp8 matmul (contraction 256/instr, 0.5 cyc/col).
    FP8 = mybir.dt.float8e4
    xts_d = [nc.dram_tensor(f"xt{i}", [C, T], FP8, kind="ExternalInput")
             for i in range(3)]
    wqs_d = [nc.dram_tensor(f"wq{i}", [C, 512], FP8, kind="ExternalInput")
             for i in range(3)]
    wks_d = [nc.dram_tensor(f"wk{i}", [C, 512], FP8, kind="ExternalInput")
             for i in range(3)]
    wvs_d = [nc.dram_tensor(f"wv{i}", [C, 512], FP8, kind="ExternalInput")
             for i in range(3)]
    bqk_t = nc.dram_tensor("bqk_t", [128, 8], F32, kind="ExternalInput")
    bv = nc.dram_tensor("bv", [1, 512], BF16, kind="ExternalInput")
    wp = nc.dram_tensor("wp", [512, C], BF16, kind="ExternalInput")
    out = nc.dram_tensor("out", [T, C], BF16, kind="ExternalOutput")

    with tile.TileContext(nc) as tc:
        with (
            tc.tile_pool(name="consts", bufs=1) as consts,
            tc.tile_pool(name="xtp", bufs=1) as xtp,
            tc.tile_pool(name="wvp", bufs=1) as wvp,
            tc.tile_pool(name="w8p", bufs=1) as w8p,
            tc.tile_pool(name="wpp", bufs=1) as wpp,
            tc.tile_pool(name="vp", bufs=1) as vp,
            tc.tile_pool(name="qkp", bufs=1) as qkp,
            tc.tile_pool(name="ytp", bufs=1) as ytp,
            tc.tile_pool(name="ptp", bufs=6) as ptp,
            tc.tile_pool(name="rcpp", bufs=2) as rcpp,
            tc.tile_pool(name="ytup", bufs=2) as ytup,
            tc.tile_pool(name="outp", bufs=3) as outp,
            tc.tile_pool(name="ps", bufs=1, space="PSUM") as ps,
        ):
            # ================= DMA staging (emission order = priority) ======
            # Transfers run FIFO in descriptor order on the shared DMA-engine
            # pool (~350 B/ns aggregate) and descriptor gen is ~625ns serial
            # per queue — so emission order IS the transfer schedule. One SP
            # chain, ordered by when the compute consumes each piece.
            xt_sb = [xtp.tile([128, CT, T], FP8, tag=f"xt{i}",
                              name=f"xt{i}") for i in range(3)]
            xt_rs = [t.ap().rearrange("(k p) t -> p k t", p=128)
                     for t in xts_d]
            wv_sb = [wvp.tile([128, CT, 512], FP8, tag=f"wv{i}",
                              name=f"wv{i}") for i in range(3)]
            wv_rs = [t.ap().rearrange("(k p) n -> p k n", p=128)
                     for t in wvs_d]
            for i in range(3):
                nc.sync.dma_start(out=xt_sb[i][:, :, 0:128],
                                  in_=xt_rs[i][:, :, 0:128])
                nc.sync.dma_start(out=wv_sb[i][:, 0:4, :],
                                  in_=wv_rs[i][:, 0:4, :])
            for i in range(3):
                nc.sync.dma_start(out=wv_sb[i][:, 4:CT, :],
                                  in_=wv_rs[i][:, 4:CT, :])
            vecs = consts.tile([128, 640], BF16, tag="vecs")
            bv_sb = vecs[32:33, 0:512]
            nc.sync.dma_start(out=bv_sb, in_=bv[:, :])
            for (c0_, c1_) in ((128, 256), (256, 512), (512, 1024)):
                for i in range(3):
                    nc.sync.dma_start(
                        out=xt_sb[i][:, :, c0_:c1_],
                        in_=xt_rs[i][:, :, c0_:c1_]
                    )

            bqk_sb = consts.tile([128, 8], F32, tag="bqk")
            nc.sync.dma_start(out=bqk_sb, in_=bqk_t[:, :])

            # qk weights: w8[(p, side)] = 3 x [128, CT, 128] fp8 triplets
            w8_sb = {}
            for p in range(NPAIR):
                for side, wsrcs in ((0, wqs_d), (1, wks_d)):
                    trip = []
                    for i in range(3):
                        t_ = w8p.tile([128, CT, 128], FP8,
                                      tag=f"w8_{p}_{side}_{i}",
                                      name=f"w8_{p}_{side}_{i}")
                        w_src = wsrcs[i].ap().rearrange(
                            "(k pp) m -> pp k m", pp=128
                        )[:, :, p * 128:(p + 1) * 128]
                        nc.sync.dma_start(out=t_, in_=w_src)
                        trip.append(t_)
                    w8_sb[(p, side)] = trip

            # wp: one descriptor, [128, NPAIR, 1024]
            wp_sb = wpp.tile([128, NPAIR, C], BF16, tag="wp")
            nc.sync.dma_start(
                out=wp_sb, in_=wp.ap().rearrange("(k p) n -> p k n", p=128)
            )
            # xt tail columns — only needed from V(t8..15) / qk ch1 onward
            for (c0_, c1_) in ((1024, 1536), (1536, 2048)):
                for i in range(3):
                    nc.sync.dma_start(
                        out=xt_sb[i][:, :, c0_:c1_],
                        in_=xt_rs[i][:, :, c0_:c1_]
                    )

            # constants
            ones_f32 = consts.tile([128, 128], F32, tag="ones_f32")
            nc.vector.memset(ones_f32, 1.0)
            triu_f32 = consts.tile([128, 128], F32, tag="triu_f32")
            nc.gpsimd.memset(triu_f32, 1.0)
            nc.gpsimd.affine_select(
                out=triu_f32, in_=triu_f32,
                compare_op=ALU.is_ge,
                fill=0.0, base=0, pattern=[[1, 128]], channel_multiplier=-1,
            )
            triu_sb = consts.tile([128, 128], BF16, tag="triu")
            nc.vector.tensor_copy(triu_sb, triu_f32)
            # rank-1 v-bias vector ones: lhsT/rhs share base partition 32
            ones32_sb = vecs[32:33, 512:640]
            nc.vector.tensor_copy(ones32_sb, ones_f32[0:1, 0:128])

            # ================= persistent SBUF state ========================
            # v tiles: [128, 8 heads x (64 v-dims | 64 ones)]
            v_sb = []
            for t in range(KT):
                t_ = vp.tile([128, HL * 128], BF16, tag=f"v{t}", name=f"v{t}")
                v_sb.append(t_)

            def memset_v_ones(t):
                # ones columns only (strided): DVE, emitted per-tile right
                # before its V projection so 16 memsets don't jam the DVE
                # FIFO ahead of the V-phase psum copies. (gpsimd.memset on a
                # bf16 tile produced wrong bits on HW — keep this on DVE.)
                v_r = v_sb[t].rearrange("p (h x) -> p h x", h=HL)
                nc.vector.memset(v_r[:, :, 64:128], 1.0)
            qt_sb = [qkp.tile([128, T], BF16, tag=f"qt{p}", name=f"qt{p}")
                     for p in range(NPAIR)]
            kt_sb = [qkp.tile([128, T], BF16, tag=f"kt{p}", name=f"kt{p}")
                     for p in range(NPAIR)]
            yt_sb = [ytp.tile([128, T], BF16, tag=f"yt{p}", name=f"yt{p}")
                     for p in range(NPAIR)]

            # ---- PE warmup: the tensor engine ramps (1.54 -> 0.83 -> 0.42
            # ns/col over ~3us of continuous work). Burn the initial DMA wait
            # on throwaway matmuls over the freshly-memset v tile so the real
            # V projection starts at full clock.
            warm = ps.tile([128, CH], F32, tag="stp", bufs=2, name="warm")
            for i_ in range(12):
                s0 = 512 * (i_ % 2)
                nc.tensor.matmul(
                    warm[:, s0:s0 + 512],
                    lhsT=v_sb[0][:, 0:128], rhs=v_sb[0][:, 0:512],
                    start=True, stop=True,
                )

            # ================= filler queue =================================
            # The exp stream on ACT runs ~0.2us/ki slower than PE's S+AV, so
            # stalls accumulate inside a head-chunk. Queue independent PE work
            # (V tiles, out-proj qtiles) as single-matmul callables and pop
            # one between S(ki) and AV(ki-1) — PE chews filler exactly where
            # it would otherwise wait for exp(ki-1).
            from collections import deque
            fill_q = deque()

            def fill(n=1):
                for _ in range(n):
                    if not fill_q:
                        return
                    fill_q.popleft()()

            def drain_all():
                """MUST run before any direct 'small'/'ot' tile allocation:
                a queued unit left half-emitted would have its rotating psum
                buffer stolen mid-accumulation (silent corruption)."""
                while fill_q:
                    fill(1)

            def queue_v(t):
                """Enqueue V projection for key-tile t as per-op callables."""
                st = {}

                def mk_mm(i, kp):
                    def f():
                        if i == 0 and kp == 0:
                            st["pv"] = ps.tile([128, 512], F32, tag="small",
                                               bufs=2, name=f"pv{t}")
                        nc.tensor.matmul(
                            st["pv"],
                            lhsT=xt_sb[i][:, 2 * kp:2 * kp + 2,
                                          t * 128:(t + 1) * 128],
                            rhs=wv_sb[i][:, 2 * kp:2 * kp + 2, :],
                            start=(i == 0 and kp == 0), stop=False,
                            perf_mode=DR,
                        )
                    return f

                def bias():
                    nc.tensor.matmul(
                        st["pv"], lhsT=ones32_sb, rhs=bv_sb,
                        start=False, stop=True,
                    )

                def copy():
                    v_r = v_sb[t].rearrange("p (h x) -> p h x", h=HL)
                    pv_r = st["pv"].rearrange("p (h d) -> p h d", h=HL)
                    nc.vector.tensor_copy(v_r[:, :, 0:64], pv_r)

                fill_q.append(lambda: memset_v_ones(t))
                for i in range(3):
                    for kp in range(CT // 2):
                        fill_q.append(mk_mm(i, kp))
                fill_q.append(bias)
                fill_q.append(copy)

            def queue_qk(p, ch):
                """Enqueue qk projection (filler variant: copies on DVE only,
                so no exp-stream interference when popped mid-attention).
                Returns a counter dict; drain until counter hits 0 before
                emitting anything that reads qt/kt of this pair+chunk."""
                st = {}
                cnt = {"n": 0}

                def wrap(f):
                    cnt["n"] += 1

                    def g():
                        f()
                        cnt["n"] -= 1
                    return g

                def mk_mm(side, s0, i, kp):
                    def f():
                        if i == 0 and kp == 0:
                            st[(side, s0)] = ps.tile(
                                [128, 512], F32, tag="small", bufs=2,
                                name=f"pq{p}_{side}_{ch}_{s0}")
                        nc.tensor.matmul(
                            st[(side, s0)],
                            lhsT=w8_sb[(p, side)][i][:, 2 * kp:2 * kp + 2, :],
                            rhs=xt_sb[i][:, 2 * kp:2 * kp + 2,
                                         ch * CH + s0:ch * CH + s0 + 512],
                            start=(i == 0 and kp == 0),
                            stop=(i == 2 and kp == CT // 2 - 1),
                            perf_mode=DR,
                        )
                    return f

                def mk_copy(side, s0):
                    dst = qt_sb[p] if side == 0 else kt_sb[p]
                    bcol = bqk_sb[:, 4 * side + p:4 * side + p + 1]

                    def f():
                        nc.vector.tensor_scalar(
                            out=dst[:, ch * CH + s0:ch * CH + s0 + 512],
                            in0=st[(side, s0)], scalar1=bcol, scalar2=None,
                            op0=ALU.add,
                        )
                    return f

                for side in (0, 1):
                    for s0 in (0, 512):
                        for i in range(3):
                            for kp in range(CT // 2):
                                fill_q.append(wrap(mk_mm(side, s0, i, kp)))
                        fill_q.append(wrap(mk_copy(side, s0)))
                return cnt

            def queue_out(qt_i):
                """Enqueue out-projection for query tile qt_i (DVE copies)."""
                st = {}

                def mk_mm(s0, p):
                    def f():
                        if p == 0:
                            st[s0] = ps.tile([128, 512], F32, tag="small",
                                             bufs=2, name=f"pso{qt_i}_{s0}")
                        nc.tensor.matmul(
                            st[s0],
                            lhsT=yt_sb[p][:, qt_i * 128:(qt_i + 1) * 128],
                            rhs=wp_sb[:, p, s0:s0 + 512],
                            start=(p == 0), stop=(p == NPAIR - 1),
                        )
                    return f

                def mk_copy(s0):
                    def f():
                        if "ot" not in st:
                            st["ot"] = outp.tile([128, C], BF16, tag="ot",
                                                 name=f"ot{qt_i}")
                        nc.vector.tensor_copy(
                            st["ot"][:, s0:s0 + 512], st[s0]
                        )
                    return f

                def dma():
                    nc.sync.dma_start(
                        out=out.ap()[qt_i * 128:(qt_i + 1) * 128, :],
                        in_=st["ot"],
                    )

                for s0 in (0, 512):
                    for p in range(NPAIR):
                        fill_q.append(mk_mm(s0, p))
                    fill_q.append(mk_copy(s0))
                fill_q.append(dma)

            # ================= phase emitters ===============================
            DR = mybir.MatmulPerfMode.DoubleRow

            def emit_v(t):
                """V projection for key-tile t -> v_sb[t] (fp8 3-term)."""
                memset_v_ones(t)
                pv = ps.tile([128, 512], F32, tag="small", bufs=2,
                             name=f"pv{t}")
                for i in range(3):
                    for kp in range(CT // 2):
                        nc.tensor.matmul(
                            pv,
                            lhsT=xt_sb[i][:, 2 * kp:2 * kp + 2,
                                          t * 128:(t + 1) * 128],
                            rhs=wv_sb[i][:, 2 * kp:2 * kp + 2, :],
                            start=(i == 0 and kp == 0), stop=False,
                            perf_mode=DR,
                        )
                nc.tensor.matmul(
                    pv, lhsT=ones32_sb, rhs=bv_sb,
                    start=False, stop=True,
                )
                v_r = v_sb[t].rearrange("p (h x) -> p h x", h=HL)
                pv_r = pv.rearrange("p (h d) -> p h d", h=HL)
                nc.vector.tensor_copy(v_r[:, :, 0:64], pv_r)

            def emit_qk(p, ch):
                """qk projection for pair p, T-chunk ch (fp8 3-term)."""
                drain_all()
                for side, dst in ((0, qt_sb[p]), (1, kt_sb[p])):
                    w8 = w8_sb[(p, side)]
                    bcol = bqk_sb[:, 4 * side + p:4 * side + p + 1]
                    for s0 in (0, 512):
                        pq = ps.tile([128, 512], F32, tag="small", bufs=2,
                                     name=f"pq{p}_{side}_{ch}_{s0}")
                        for i in range(3):
                            for kp in range(CT // 2):
                                nc.tensor.matmul(
                                    pq,
                                    lhsT=w8[i][:, 2 * kp:2 * kp + 2, :],
                                    rhs=xt_sb[i][:, 2 * kp:2 * kp + 2,
                                                 ch * CH + s0:
                                                 ch * CH + s0 + 512],
                                    start=(i == 0 and kp == 0),
                                    stop=(i == 2 and kp == CT // 2 - 1),
                                    perf_mode=DR,
                                )
                        # q-side copies on DVE, k-side on ACT (GPSIMD cannot
                        # read PSUM): the first S matmul needs BOTH qt and kt
                        # — parallel engines halve that critical-path latency.
                        if side == 0:
                            nc.vector.tensor_scalar(
                                out=dst[:, ch * CH + s0:ch * CH + s0 + 512],
                                in0=pq, scalar1=bcol, scalar2=None,
                                op0=ALU.add,
                            )
                        else:
                            nc.scalar.activation(
                                out=dst[:, ch * CH + s0:ch * CH + s0 + 512],
                                in_=pq, func=AF.Identity, bias=bcol, scale=1.0,
                            )

            def emit_att(p, c):
                """Attention for pair p's two heads over query chunk c."""
                kmax = 8 * (c + 1)
                for hh in range(2):
                    hloc = 2 * p + hh
                    base = 64 * hh
                    qt_t, kt_t = qt_sb[p], kt_sb[p]
                    ytps = ps.tile([128, CH], F32, tag="ytps", bufs=1,
                                   name=f"ytps{hloc}_{c}")

                    def segs_of(ki):
                        q_off = max(0, 128 * ki - CH * c)
                        segs = []
                        if q_off < 512:
                            segs.append((q_off, 512))
                        segs.append((max(q_off, 512), CH))
                        return q_off, segs

                    def emit_s(ki):
                        q_off, segs = segs_of(ki)
                        stp = ps.tile([128, CH], F32, tag="stp", bufs=2,
                                      name=f"stp{hloc}_{c}_{ki}")
                        for (s0, s1) in segs:
                            nc.tensor.matmul(
                                stp[:, s0:s1],
                                lhsT=kt_t[base:base + 64,
                                          ki * 128:(ki + 1) * 128],
                                rhs=qt_t[base:base + 64,
                                         CH * c + s0:CH * c + s1],
                                start=True, stop=True,
                            )
                        pt = ptp.tile([128, CH], BF16, tag="pt",
                                      name=f"pt{hloc}_{c}_{ki}")
                        nc.scalar.activation(
                            out=pt[:, q_off:CH], in_=stp[:, q_off:CH],
                            func=AF.Exp, scale=SCALE,
                        )
                        if ki >= 8 * c:  # causal mask on diagonal block
                            nc.vector.tensor_mul(
                                pt[:, q_off:q_off + 128],
                                pt[:, q_off:q_off + 128], triu_sb,
                            )
                        return pt

                    b0_last = min(kmax - 1, 8 * c + 3)

                    def emit_av(ki, pt):
                        q_off, segs = segs_of(ki)
                        for (s0, s1) in segs:
                            last = b0_last if s0 < 512 else kmax - 1
                            nc.tensor.matmul(
                                ytps[:, s0:s1],
                                lhsT=v_sb[ki][:, 128 * hloc:128 * hloc + 128],
                                rhs=pt[:, s0:s1],
                                start=(ki == 0), stop=(ki == last),
                            )

                    # 2-deep software pipeline: S(0),S(1),S(2) precede
                    # AV(0), giving each chunk boundary ~2.5us of PE runway
                    # to cover the previous chunk's ytps drain chain (DVE
                    # rcp+mul, ~2.6us). Fillers pop where PE would wait for
                    # exp(ki-2).
                    pts = {0: emit_s(0)}
                    if kmax > 1:
                        pts[1] = emit_s(1)
                    for ki in range(2, kmax):
                        pts[ki] = emit_s(ki)
                        fill(1)
                        emit_av(ki - 2, pts.pop(ki - 2))
                    fill(1)
                    emit_av(kmax - 2, pts.pop(kmax - 2))
                    emit_av(kmax - 1, pts.pop(kmax - 1))

                    # normalize: y * (1/den). den sits on psum rows 64:127.
                    # Engine lanes are partition-locked: every compute op
                    # stays partition-aligned; the 64->0 partition move is an
                    # SBUF->SBUF DMA (engine-free, HW-verified pattern).
                    # Drain ytps through TWO engines in parallel — DVE
                    # reciprocal of den (rows 64:128) and ACT Identity copy
                    # of y (rows 0:64) — so ytps frees in ~1.3us (< the next
                    # head-chunk's S(0)+S(1) PE time); the DMA shift and the
                    # all-SBUF bf16 multiply trail off the critical path.
                    # DVE ops map lanes RELATIVELY within the partition
                    # range (the fp32r baseline did a cross-partition
                    # reciprocal on HW), so read den from rows 64:128 and
                    # write 1/den to rows 0:64 directly — no DMA shift, no
                    # ACT staging. Per column half so consumers (next
                    # head-chunk / out-proj) unblock sooner.
                    rcp_sb = rcpp.tile([64, CH], BF16, tag="rcp",
                                       name=f"rcp{hloc}_{c}")
                    if p == NPAIR - 1 and c == 1 and hh == 1:
                        # last head-chunk: quarter granularity so the tail
                        # out-projection's pair-3 matmuls unblock asap
                        nspans = tuple((q * 256, q * 256 + 256)
                                       for q in range(4))
                    else:
                        nspans = ((0, 512), (512, CH))
                    for (d0, d1) in nspans:
                        with nc.allow_low_precision(
                            reason="1/denominator in bf16: 0.4% rel on a "
                                   "well-conditioned positive sum, budget 2e-2"
                        ):
                            nc.vector.reciprocal(
                                out=rcp_sb[:, d0:d1],
                                in_=ytps[64:128, d0:d1],
                            )
                        nc.vector.tensor_mul(
                            yt_sb[p][base:base + 64, CH * c + d0:CH * c + d1],
                            ytps[0:64, d0:d1], rcp_sb[:, d0:d1],
                        )

            def emit_out(qt_i, act_halves=(), split_dma=False):
                """Output projection for query tile qt_i + DMA to dram.

                act_halves: column halves whose psum->sbuf copy goes to the
                Activation engine — only safe once attention exp work there
                is done (ACT otherwise delays the exp stream).
                """
                drain_all()
                ot = outp.tile([128, C], BF16, tag="ot", name=f"ot{qt_i}")
                for s0 in (0, 512):
                    pso = ps.tile([128, 512], F32, tag="small", bufs=2,
                                  name=f"pso{qt_i}_{s0}")
                    for p in range(NPAIR):
                        nc.tensor.matmul(
                            pso,
                            lhsT=yt_sb[p][:, qt_i * 128:(qt_i + 1) * 128],
                            rhs=wp_sb[:, p, s0:s0 + 512],
                            start=(p == 0), stop=(p == NPAIR - 1),
                        )
                    if split_dma == "quarters":
                        # very last tile: quarter copies alternating DVE/ACT
                        # with immediate DMAs, so the post-final-matmul serial
                        # chain (copy -> desc-gen -> DMA -> sem) is minimal
                        for qq in (0, 256):
                            col = s0 + qq
                            if (qq // 256) % 2 == 0:
                                nc.vector.tensor_copy(
                                    ot[:, col:col + 256],
                                    pso[:, qq:qq + 256],
                                )
                            else:
                                nc.scalar.activation(
                                    out=ot[:, col:col + 256],
                                    in_=pso[:, qq:qq + 256],
                                    func=AF.Copy, scale=1.0,
                                )
                            nc.sync.dma_start(
                                out=out.ap()[qt_i * 128:(qt_i + 1) * 128,
                                             col:col + 256],
                                in_=ot[:, col:col + 256],
                            )
                        continue
                    if s0 in act_halves:
                        nc.scalar.activation(
                            out=ot[:, s0:s0 + 512], in_=pso,
                            func=AF.Copy, scale=1.0,
                        )
                    else:
                        nc.vector.tensor_copy(ot[:, s0:s0 + 512], pso)
                    if split_dma:  # final tiles: drain each half immediately
                        nc.sync.dma_start(
                            out=out.ap()[qt_i * 128:(qt_i + 1) * 128,
                                         s0:s0 + 512],
                            in_=ot[:, s0:s0 + 512],
                        )
                if not split_dma:
                    nc.sync.dma_start(
                        out=out.ap()[qt_i * 128:(qt_i + 1) * 128, :], in_=ot
                    )

            # ================= schedule =====================================
            for t in range(8):
                emit_v(t)
            for p in range(NPAIR):
                emit_qk(p, 0)
                # V tiles 8..15 become intra-attention fillers; pair 0 gets
                # none (its xt tail columns are still in flight on DMA).
                if p >= 1:
                    queue_v(6 + 2 * p)
                    queue_v(7 + 2 * p)
                emit_att(p, 0)
            queue_v(14)
            queue_v(15)
            while fill_q:  # V(14), V(15) + anything the slots didn't absorb
                fill(1)
            # Filler distribution for the c1 pass (out-proj qtiles 0..7 only
            # need chunk-0 data, so they can pop under ANY c1 pair): att(3,1)
            # gets out(4..7) — it is the only segment with no independent
            # block work around it, and its leftovers drain right where the
            # tail out-projection would otherwise wait on the last normalize.
            qk3_cnt = None
            for p in range(NPAIR):
                if p < NPAIR - 1:
                    emit_qk(p, 1)
                else:
                    # qk(3,ch1) was queued into att(2,c1); make sure every
                    # one of its ops is emitted before att(3,c1) reads qt/kt
                    while qk3_cnt["n"] > 0:
                        fill(1)
                if p == 0:
                    queue_out(0)
                    queue_out(1)
                elif p == 1:
                    queue_out(2)
                    queue_out(3)
                elif p == NPAIR - 2:
                    qk3_cnt = queue_qk(NPAIR - 1, 1)
                else:
                    for qt_i in range(4, 8):
                        queue_out(qt_i)
                emit_att(p, 1)
            while fill_q:
                fill(1)
            for qt_i in range(8, KT):
                # alternate whole-qtile copy engine so neither DVE nor ACT
                # serializes the tail
                halves = (0, 512) if qt_i % 2 else ()
                emit_out(qt_i, act_halves=halves, split_dma=True)

    nc.compile()
    _prog_cache[key] = nc
    return nc


def shard_inputs(x, W_qkv, b_qkv, W_proj, core):
    b, g = core // 2, core % 2
    cq = slice(512 * g, 512 * g + 512)
    ck = slice(1024 + 512 * g, 1024 + 512 * g + 512)
    cv = slice(2048 + 512 * g, 2048 + 512 * g + 512)
    def trip(a, name, weight):
        # x*w = x8*w8 + (x8/16)*(16rw) + (16rx)*(w8/16). Term i multiplies
        # array i of both operands, so the x side ships [x8, x8/16, 16rx]
        # and the weight side ships [w8, 16rw, w8/16].
        a = np.ascontiguousarray(a, dtype=np.float32)
        a8 = a.astype(NPF8)
        a8f = a8.astype(np.float32)
        shifted = (a8f / 16.0).astype(NPF8)
        resid = (16.0 * (a - a8f)).astype(NPF8)
        if weight:
            return {f"{name}0": a8, f"{name}1": resid, f"{name}2": shifted}
        return {f"{name}0": a8, f"{name}1": shifted, f"{name}2": resid}

    return {
        **trip(x[b].T, "xt", False),
        **trip(W_qkv[:, cq], "wq", True),
        **trip(W_qkv[:, ck], "wk", True),
        **trip(W_qkv[:, cv], "wv", True),
        "bqk_t": np.stack(
            [b_qkv[cq].reshape(4, 128)[p_] for p_ in range(4)]
            + [b_qkv[ck].reshape(4, 128)[p_] for p_ in range(4)], axis=1
        ).astype(np.float32).copy(),
        "bv": np.ascontiguousarray(b_qkv[cv]).reshape(1, 512).astype(NPBF16),
        "wp": np.ascontiguousarray(W_proj[512 * g:512 * g + 512, :]).astype(NPBF16),
    }


def kernel(x, W_qkv, b_qkv, W_proj, b_proj, **run_kwargs):
    x = np.asarray(x, np.float32)
    W_qkv = np.asarray(W_qkv, np.float32)
    b_qkv = np.asarray(b_qkv, np.float32)
    W_proj = np.asarray(W_proj, np.float32)
    b_proj = np.asarray(b_proj, np.float32)

    nc = build_program()
    in_maps = [
        shard_inputs(x, W_qkv, b_qkv, W_proj, core) for core in range(NCORES)
    ]
    from concourse.bass_utils import run_bass_kernel_spmd

    res = run_bass_kernel_spmd(nc, in_maps, core_ids=list(range(NCORES)), **run_kwargs)
    outs = [np.asarray(r["out"], np.float32) for r in res.results]
    full = np.stack([outs[2 * b_] + outs[2 * b_ + 1] + b_proj for b_ in range(B)])
    kernel.last_results = res
    return full
